# revision 8
# baseline (speedup 1.0000x reference)
"""EuclideanCodebook (VQ) kernel for 8x Trainium2 NeuronCores.

Problem: x [16, 4096, 256] f32, embed [1, 2048, 256] f32.
  dist(n,c) = -||x_n - e_c||;  ind = argmax_c dist (== argmin distance),
  quantize = embed[ind].  Returns (quantize [16,4096,256] f32, ind [16,4096] i32).

Strategy (data parallel, 8 cores x 8192 tokens):
  argmin_c ||x-e_c||^2 == argmax_c (2*x.e_c - ||e_c||^2)   (x^2 term constant per token)
  score[128 tok, 2048 codes] accumulated in PSUM by 7 fp16 matmuls:
    x, 2e split hi/lo into fp16 (exact 11-bit significands; PE fp16 products
    are exact, fp32 accumulate) -> fp32-grade scores:
      xhi.ehi (k0,k1) + xlo.ehi (k0,k1) + xhi.elo (k0,k1) + ones3.(-e2 split into 3 fp16 rows)
  DVE max + max_index -> argmax index per token; indirect DMA gathers
  embed rows from HBM; results DMA'd out.
"""

import numpy as np

import concourse.bass as bass
import concourse.mybir as mybir
from concourse import bacc
from concourse.tile import TileContext
from concourse.bass_utils import run_bass_kernel_spmd

B, N, D = 16, 4096, 256
C = 2048              # codebook size
NCORES = 8
TOK = (B * N) // NCORES     # 8192 tokens per core
P = 128
KC = D // P                 # 2 contraction chunks
MB = TOK // P               # 64 token blocks per core
NB = C // 512               # 4 code blocks (psum banks)
F16 = mybir.dt.float16
F32 = mybir.dt.float32
U32 = mybir.dt.uint32
I32 = mybir.dt.int32

_CACHE = {}


def _build_nc(xch=16, psum_bufs=2, gat_bufs=4):
    nc = bacc.Bacc("TRN2")

    t_xhi = nc.dram_tensor("xhi", [P, KC, TOK], F16, kind="ExternalInput")
    t_xlo = nc.dram_tensor("xlo", [P, KC, TOK], F16, kind="ExternalInput")
    t_ehi = nc.dram_tensor("ehi", [P, KC, C], F16, kind="ExternalInput")
    t_elo = nc.dram_tensor("elo", [P, KC, C], F16, kind="ExternalInput")
    t_e2n = nc.dram_tensor("e2n", [3, C], F16, kind="ExternalInput")
    t_ones = nc.dram_tensor("ones3", [3, P], F16, kind="ExternalInput")
    t_emb = nc.dram_tensor("emb", [C, D], F32, kind="ExternalInput")

    t_q = nc.dram_tensor("q", [TOK, D], F32, kind="ExternalOutput")
    t_ind = nc.dram_tensor("ind", [TOK], I32, kind="ExternalOutput")

    q_view = t_q.ap().rearrange("(mb p) d -> mb p d", p=P)
    ind_view = t_ind.ap().rearrange("(mb p) -> p mb", p=P)

    with TileContext(nc) as tc:
        with (
            tc.tile_pool(name="const", bufs=1) as cpool,
            tc.tile_pool(name="psum", bufs=psum_bufs, space="PSUM") as psum,
            tc.tile_pool(name="m8", bufs=2) as m8pool,
            tc.tile_pool(name="gat", bufs=gat_bufs) as gpool,
        ):
            s_xhi = cpool.tile([P, KC, TOK], F16)
            s_xlo = cpool.tile([P, KC, TOK], F16)
            s_ehi = cpool.tile([P, KC, C], F16)
            s_elo = cpool.tile([P, KC, C], F16)
            s_e2n = cpool.tile([3, C], F16)
            s_ones = cpool.tile([3, P], F16)
            s_ind8 = cpool.tile([P, MB, 8], U32)

            nc.sync.dma_start(s_e2n[:], t_e2n[:])
            nc.sync.dma_start(s_ones[:], t_ones[:])
            nc.sync.dma_start(s_ehi[:], t_ehi[:])
            # x in chunks so PE can start early; elo interleaved after the
            # first x chunk (only needed by the second lhsT group)
            XCH = TOK // xch
            for ci in range(xch):
                sl = slice(ci * XCH, (ci + 1) * XCH)
                nc.sync.dma_start(s_xhi[:, :, sl], t_xhi[:, :, sl])
                nc.sync.dma_start(s_xlo[:, :, sl], t_xlo[:, :, sl])
                if ci == 0:
                    nc.sync.dma_start(s_elo[:], t_elo[:])

            for mb in range(MB):
                msl = slice(mb * P, (mb + 1) * P)
                score = psum.tile([P, C], F32, name="score")
                # (lhsT, [rhs list]) in LDWEIGHTS-friendly order
                groups = []
                for k in range(KC):
                    groups.append((s_xhi[:, k, msl], [s_ehi[:, k], s_elo[:, k]]))
                for k in range(KC):
                    groups.append((s_xlo[:, k, msl], [s_ehi[:, k]]))
                groups.append((s_ones[:], [s_e2n[:]]))
                n_mm_per_bank = sum(len(r) for _, r in groups)  # 7
                bank_count = [0] * NB
                for lhsT, rhss in groups:
                    for rhs in rhss:
                        for nb in range(NB):
                            nsl = slice(nb * 512, (nb + 1) * 512)
                            bank_count[nb] += 1
                            nc.tensor.matmul(
                                score[:, nsl], lhsT, rhs[:, nsl],
                                start=(bank_count[nb] == 1),
                                stop=(bank_count[nb] == n_mm_per_bank),
                            )
                max8 = m8pool.tile([P, 8], F32, name="max8")
                nc.vector.max(out=max8[:], in_=score[:])
                nc.vector.max_index(
                    out=s_ind8[:, mb, :], in_max=max8[:], in_values=score[:]
                )
                gat = gpool.tile([P, D], F32, name="gat")
                nc.gpsimd.indirect_dma_start(
                    out=gat[:],
                    out_offset=None,
                    in_=t_emb[:],
                    in_offset=bass.IndirectOffsetOnAxis(
                        ap=s_ind8[:, mb, :1], axis=0
                    ),
                )
                nc.sync.dma_start(q_view[mb], gat[:])

            nc.sync.dma_start(ind_view[:], s_ind8[:, :, 0].bitcast(I32))

    nc.finalize()
    return nc


def _fp16_split(a32):
    hi = a32.astype(np.float16)
    lo = (a32 - hi.astype(np.float32)).astype(np.float16)
    return hi, lo


def _prepare(x, embed):
    x = np.asarray(x, dtype=np.float32)
    embed = np.asarray(embed, dtype=np.float32)
    e = embed[0]                                   # [C, D]

    if "nc" not in _CACHE:
        _CACHE["nc"] = _build_nc()
    nc = _CACHE["nc"]

    # ---- host-side prep (layouts for the device) ----
    e2x = 2.0 * e                                  # [C, D]
    et = np.ascontiguousarray(e2x.T)               # [D, C]
    ehi, elo = _fp16_split(et)
    ehi = np.ascontiguousarray(
        ehi.reshape(KC, P, C).transpose(1, 0, 2))  # [P, KC, C]
    elo = np.ascontiguousarray(
        elo.reshape(KC, P, C).transpose(1, 0, 2))

    e2 = np.sum(e * e, axis=1, dtype=np.float32)   # [C]
    r = (-e2).astype(np.float32)
    rows = []
    for _ in range(3):
        h = r.astype(np.float16)
        rows.append(h)
        r = r - h.astype(np.float32)
    e2n = np.stack(rows, axis=0)                   # [3, C] fp16
    ones3 = np.ones((3, P), dtype=np.float16)

    xf = x.reshape(B * N, D)
    in_maps = []
    for c in range(NCORES):
        shard = xf[c * TOK:(c + 1) * TOK]          # [TOK, D]
        xt = np.ascontiguousarray(shard.T)         # [D, TOK]
        xhi, xlo = _fp16_split(xt)
        xhi = np.ascontiguousarray(
            xhi.reshape(KC, P, TOK).transpose(1, 0, 2))  # [P, KC, TOK]
        xlo = np.ascontiguousarray(
            xlo.reshape(KC, P, TOK).transpose(1, 0, 2))
        in_maps.append({
            "xhi": xhi, "xlo": xlo,
            "ehi": ehi, "elo": elo,
            "e2n": e2n, "ones3": ones3,
            "emb": e,
        })
    return nc, in_maps


def kernel(x, embed):
    nc, in_maps = _prepare(x, embed)
    res = run_bass_kernel_spmd(nc, in_maps, core_ids=list(range(NCORES)))

    quant = np.concatenate(
        [res.results[c]["q"] for c in range(NCORES)], axis=0
    ).reshape(B, N, D)
    ind = np.concatenate(
        [res.results[c]["ind"] for c in range(NCORES)], axis=0
    ).reshape(B, N).astype(np.int32)
    return quant, ind

